# revision 11
# baseline (speedup 1.0000x reference)
"""Trainium2 Bass kernel for the CustomLSTM problem.

Problem: B=2048, T=256, I=5, H=50, O=1 LSTM; y = fc(h_T).

Strategy (data-parallel over batch, 8 cores x 256):
  - State kept transposed: hT/c are [H=50, B_local] (hidden on partitions,
    batch on the free dim) so the recurrent matmul needs no transposes.
  - Per step, gates are computed as two M=128 matmuls into one PSUM tile
    [128, 512]:  cols 0:256 = [f(0:50); i(64:114)],
                 cols 256:512 = [o(0:50); 2*g_c(64:114)]  (pads zero).
    rhs = [h; x_t; 1] (K=56) folds both the input projection and all
    biases into the same matmul (bias row = constant 1).
    The 64-offset satisfies the engine rule that SBUF access patterns
    start at partition 0/32/64/96.
  - The c-gate weights are pre-doubled so ONE Sigmoid activation over
    [128, 512] yields F, I, O and U = sigmoid(2 g_c); tanh(g_c) = 2U-1.
  - c update: P = [F;I] * [c; 2U-1] elementwise (DVE), then the
    cross-partition sum  c' = P[0:50] + P[64:114]  is done on the PE via a
    matmul with a stacked-identity lhsT (engines cannot cross partitions).
  - h update: V = tanh(c') (ACT, same table set as Sigmoid), h = O * V.
  - Final fc is one K=56 matmul with Wfc/bias folded the same way.
"""

import numpy as np

B, T, IN, H, OUT = 2048, 256, 5, 50, 1
NCORES = 8
BL = B // NCORES  # 256 batch per core
K = H + IN + 1  # 56: [h; x; 1]
H2 = 64  # partition offset of the second gate in each pair
M = 128  # matmul output partitions (f/o at 0:50, i/c at 64:114)
NR = 4  # rhs ring buffers

# dtype config: "f32" (pure fp32) or "f16" (fp16 SBUF tensors, fp32 PSUM)
SB_DT = "f16"


def _np_dt():
    return np.float16 if SB_DT == "f16" else np.float32


def _build_weights(inp, np_dt):
    Whf, Whi, Whc, Who = inp["Whf"], inp["Whi"], inp["Whc"], inp["Who"]
    Wxf, Wxi, Wxc, Wxo = inp["Wxf"], inp["Wxi"], inp["Wxc"], inp["Wxo"]
    b_f = inp["bxf"] + inp["bhf"] + inp["bf"]
    b_i = inp["bxi"] + inp["bhi"] + inp["bi"]
    b_c = inp["bxc"] + inp["bhc"] + inp["bc"]
    b_o = inp["bxo"] + inp["bho"] + inp["bo"]

    def pack(Wh_a, Wx_a, b_a, Wh_b, Wx_b, b_b, scale_b=1.0):
        W = np.zeros((K, M), dtype=np.float64)
        W[0:H, 0:H] = Wh_a.T
        W[H : H + IN, 0:H] = Wx_a.T
        W[H + IN, 0:H] = b_a
        W[0:H, H2 : H2 + H] = scale_b * Wh_b.T
        W[H : H + IN, H2 : H2 + H] = scale_b * Wx_b.T
        W[H + IN, H2 : H2 + H] = scale_b * b_b
        return W

    W_fi = pack(Whf, Wxf, b_f, Whi, Wxi, b_i)
    W_co = pack(Who, Wxo, b_o, Whc, Wxc, b_c, scale_b=2.0)

    Ired = np.zeros((M, H2), dtype=np.float64)
    Ired[0:H, 0:H] = np.eye(H)
    Ired[H2 : H2 + H, 0:H] = np.eye(H)

    W_fc = np.zeros((K, OUT), dtype=np.float64)
    W_fc[0:H, 0] = inp["Wfc"][0]
    W_fc[H + IN, 0] = inp["bfc"][0]

    return (
        np.ascontiguousarray(W_fi, dtype=np_dt),
        np.ascontiguousarray(W_co, dtype=np_dt),
        np.ascontiguousarray(Ired, dtype=np_dt),
        np.ascontiguousarray(W_fc, dtype=np_dt),
    )


def _build_bass(T=T):
    import concourse.mybir as mybir
    from concourse import bacc, tile

    f32 = mybir.dt.float32
    sb = mybir.dt.float16 if SB_DT == "f16" else mybir.dt.float32
    AF = mybir.ActivationFunctionType
    ALU = mybir.AluOpType

    nc = bacc.Bacc(None)

    xT_d = nc.dram_tensor("xT", [T, IN, BL], sb, kind="ExternalInput")
    ones_d = nc.dram_tensor("ones", [1, BL], sb, kind="ExternalInput")
    wfi_d = nc.dram_tensor("w_fi", [K, M], sb, kind="ExternalInput")
    wco_d = nc.dram_tensor("w_co", [K, M], sb, kind="ExternalInput")
    ired_d = nc.dram_tensor("i_red", [M, H2], sb, kind="ExternalInput")
    wfc_d = nc.dram_tensor("w_fc", [K, OUT], sb, kind="ExternalInput")
    out_d = nc.dram_tensor("out", [OUT, BL], f32, kind="ExternalOutput")

    with tile.TileContext(nc) as tc:
        with (
            tc.tile_pool(name="const", bufs=1) as cpool,
            tc.tile_pool(name="state", bufs=1) as spool,
            tc.tile_pool(name="work", bufs=3) as wpool,
            tc.tile_pool(name="psum", bufs=3, space="PSUM") as pg_pool,
            tc.tile_pool(name="psum_c", bufs=2, space="PSUM") as pc_pool,
            tc.tile_pool(name="psum_fc", bufs=1, space="PSUM") as pfc_pool,
        ):
            # constants
            wfi_sb = cpool.tile([K, M], sb, tag="wfi")
            wco_sb = cpool.tile([K, M], sb, tag="wco")
            ired_sb = cpool.tile([M, H2], sb, tag="ired")
            wfc_sb = cpool.tile([K, OUT], sb, tag="wfc")
            nc.sync.dma_start(wfi_sb[:], wfi_d[:])
            nc.sync.dma_start(wco_sb[:], wco_d[:])
            nc.sync.dma_start(ired_sb[:], ired_d[:])
            nc.sync.dma_start(wfc_sb[:], wfc_d[:])

            # persistent state
            # S: rows 0:50 = c, rows 64:114 = c~ scratch, pads stay zero
            S = spool.tile([M, BL], sb, tag="S")
            nc.vector.memset(S[:], 0.0)
            # rhs ring: [h(0:50); x(50:55); ones(55)]
            R = [
                spool.tile([K, BL], sb, tag=f"R{i}", name=f"R{i}")
                for i in range(NR)
            ]
            for i in range(NR):
                nc.sync.dma_start(R[i][H + IN : K, :], ones_d[:])
            nc.vector.memset(R[0][0:H, :], 0.0)

            for t in range(T):
                r = R[t % NR]
                # prefetch x_t into the rhs ring (DMA is exempt from the
                # partition-start rule)
                nc.sync.dma_start(r[H : H + IN, :], xT_d[t])

                pg = pg_pool.tile([M, 2 * BL], f32, tag="pg")
                nc.tensor.matmul(
                    pg[:, 0:BL], wfi_sb[:], r[:], start=True, stop=True
                )
                nc.tensor.matmul(
                    pg[:, BL : 2 * BL], wco_sb[:], r[:], start=True, stop=True
                )

                # G: [F; I] cols 0:BL, [O; U] cols BL:2BL
                G = wpool.tile([M, 2 * BL], sb, tag="G")
                nc.scalar.activation(G[:], pg[:], AF.Sigmoid)

                # c~ = 2U - 1 into S rows 64:114
                nc.vector.tensor_scalar(
                    out=S[H2 : H2 + H, :],
                    in0=G[H2 : H2 + H, BL : 2 * BL],
                    scalar1=2.0,
                    scalar2=1.0,
                    op0=ALU.mult,
                    op1=ALU.subtract,
                )
                # P = [F; I] * [c; c~]   (pad rows: G*0 = 0)
                P = wpool.tile([M, BL], sb, tag="P")
                nc.vector.tensor_mul(P[:], G[:, 0:BL], S[:])

                # c' = P[0:50] + P[64:114] via PE reduction
                pc = pc_pool.tile([H2, BL], f32, tag="pc")
                nc.tensor.matmul(pc[:], ired_sb[:], P[:], start=True, stop=True)

                # V = tanh(c'); c (sbuf) = c'
                V = wpool.tile([H, BL], sb, tag="V")
                nc.scalar.activation(V[:], pc[0:H, :], AF.Tanh)
                nc.vector.tensor_copy(S[0:H, :], pc[0:H, :])

                # h = O * V into next rhs slot
                rn = R[(t + 1) % NR]
                nc.vector.tensor_mul(rn[0:H, :], G[0:H, BL : 2 * BL], V[:])

            # final fc: out = Wfc @ h + bfc (bias folded via ones row; x rows
            # of wfc are zero so stale x in the ring slot is harmless)
            pfc = pfc_pool.tile([OUT, BL], f32, tag="pfc")
            nc.tensor.matmul(pfc[:], wfc_sb[:], R[T % NR][:], start=True, stop=True)
            out_sb = wpool.tile([OUT, BL], f32, tag="osb")
            nc.scalar.copy(out=out_sb[:], in_=pfc[:])
            nc.sync.dma_start(out_d[:], out_sb[:])

    nc.compile()
    return nc


_NC_CACHE = None


def _get_nc():
    global _NC_CACHE
    if _NC_CACHE is None:
        _NC_CACHE = _build_bass()
    return _NC_CACHE


def kernel(**inputs):
    from concourse.bass_utils import run_bass_kernel_spmd

    np_dt = _np_dt()
    inp = {k: np.asarray(v, dtype=np.float32) for k, v in inputs.items()}
    W_fi, W_co, Ired, W_fc = _build_weights(inp, np_dt)

    in_maps = []
    for k in range(NCORES):
        xs = inp["x"][k * BL : (k + 1) * BL]  # [BL, T, IN]
        xT = np.ascontiguousarray(np.transpose(xs, (1, 2, 0)), dtype=np_dt)
        in_maps.append(
            {
                "xT": xT,
                "ones": np.ones((1, BL), dtype=np_dt),
                "w_fi": W_fi,
                "w_co": W_co,
                "i_red": Ired,
                "w_fc": W_fc,
            }
        )

    nc = _get_nc()
    res = run_bass_kernel_spmd(nc, in_maps, list(range(NCORES)))
    outs = [res.results[k]["out"].reshape(BL, OUT) for k in range(NCORES)]
    return np.concatenate(outs, axis=0).astype(np.float32)


# revision 13
# speedup vs baseline: 713.3939x; 713.3939x over previous
"""Trainium2 Bass kernel for the CustomLSTM problem.

Problem: B=2048, T=256, I=5, H=50, O=1 LSTM; y = fc(h_T).

Strategy (data-parallel over batch: 8 cores x 256, and CHAINS=2
independent column-group scans per core whose serial per-step dependency
chains interleave on the engines to hide latency):

  - State kept transposed: hT/c are [H=50, B_chain] (hidden on
    partitions, batch on the free dim) so the recurrent matmul needs no
    per-step transposes.
  - Per step+chain, gates are computed by two M=128 matmuls into one
    PSUM tile [128, 2*BW]:  cols 0:BW = [f(0:50); i(64:114)],
    cols BW:2BW = [o(0:50); 2*g_c(64:114)]  (pads zero).  The 64-offset
    satisfies the HW rule that engine SBUF access patterns start at
    partition 0/32/64/96.
  - rhs = [h(0:50); x_t(50:55); 1(55)] (K=56) folds the input projection
    AND all biases into the same matmul (bias column of lhsT multiplies
    the constant-1 row).  x_t is DMA'd straight into the rhs ring slot
    (DMA is exempt from the partition-start rule).
  - The c-gate weights are pre-doubled so ONE Sigmoid activation over
    [128, 2*BW] yields F, I, O and U = sigmoid(2 g_c); then
    tanh(g_c) = 2U-1 is one DVE tensor_scalar (sigmoid and tanh share an
    ACT table set, but the single-func trick saves a whole ACT op).
  - c update: P = [F;I] * [c; 2U-1] elementwise (DVE), then the
    cross-partition sum  c' = P[0:50] + P[64:114]  is done on the PE via
    a matmul with a stacked-identity lhsT (compute engines cannot move
    data across partitions).
  - h update: V = tanh(c') from PSUM (ACT), h = O * V written into the
    next rhs ring slot.  The SBUF copy of c' is emitted AFTER the
    h-multiply so the in-order DVE does not stall the critical path.
  - Everything in SBUF is fp16 (PE matmuls run at bf16 speed, DVE gets
    2x/4x modes); PSUM accumulation stays fp32.  End-to-end error vs the
    fp32 reference is ~9e-4 (validated in sim and on hardware).
  - Final fc is one K=56 matmul per chain with Wfc/bias folded the same
    way.
"""

import numpy as np

B, T, IN, H, OUT = 2048, 256, 5, 50, 1
NCORES = 8
BL = B // NCORES  # 256 batch per core
K = H + IN + 1  # 56: [h; x; 1]
H2 = 64  # partition offset of the second gate in each pair
M = 128  # matmul output partitions (f/o at 0:50, i/c at 64:114)
NR = 4  # rhs ring buffers
CHAINS = 2  # independent per-core scan chains (must divide BL)
SB_DT = "f16"


def _np_dt():
    return np.float16 if SB_DT == "f16" else np.float32


def _build_weights(inp, np_dt):
    Whf, Whi, Whc, Who = inp["Whf"], inp["Whi"], inp["Whc"], inp["Who"]
    Wxf, Wxi, Wxc, Wxo = inp["Wxf"], inp["Wxi"], inp["Wxc"], inp["Wxo"]
    b_f = inp["bxf"] + inp["bhf"] + inp["bf"]
    b_i = inp["bxi"] + inp["bhi"] + inp["bi"]
    b_c = inp["bxc"] + inp["bhc"] + inp["bc"]
    b_o = inp["bxo"] + inp["bho"] + inp["bo"]

    def pack(Wh_a, Wx_a, b_a, Wh_b, Wx_b, b_b, scale_b=1.0):
        W = np.zeros((K, M), dtype=np.float64)
        W[0:H, 0:H] = Wh_a.T
        W[H : H + IN, 0:H] = Wx_a.T
        W[H + IN, 0:H] = b_a
        W[0:H, H2 : H2 + H] = scale_b * Wh_b.T
        W[H : H + IN, H2 : H2 + H] = scale_b * Wx_b.T
        W[H + IN, H2 : H2 + H] = scale_b * b_b
        return W

    W_fi = pack(Whf, Wxf, b_f, Whi, Wxi, b_i)
    W_co = pack(Who, Wxo, b_o, Whc, Wxc, b_c, scale_b=2.0)

    Ired = np.zeros((M, H2), dtype=np.float64)
    Ired[0:H, 0:H] = np.eye(H)
    Ired[H2 : H2 + H, 0:H] = np.eye(H)

    W_fc = np.zeros((K, OUT), dtype=np.float64)
    W_fc[0:H, 0] = inp["Wfc"][0]
    W_fc[H + IN, 0] = inp["bfc"][0]

    return (
        np.ascontiguousarray(W_fi, dtype=np_dt),
        np.ascontiguousarray(W_co, dtype=np_dt),
        np.ascontiguousarray(Ired, dtype=np_dt),
        np.ascontiguousarray(W_fc, dtype=np_dt),
    )


def _build_bass(T=T, chains=CHAINS):
    import concourse.mybir as mybir
    from concourse import bacc, tile

    f32 = mybir.dt.float32
    sb = mybir.dt.float16 if SB_DT == "f16" else mybir.dt.float32
    AF = mybir.ActivationFunctionType
    ALU = mybir.AluOpType

    BW = BL // chains
    assert BW * chains == BL

    nc = bacc.Bacc(None)

    xT_d = nc.dram_tensor("xT", [T, IN, BL], sb, kind="ExternalInput")
    ones_d = nc.dram_tensor("ones", [1, BL], sb, kind="ExternalInput")
    wfi_d = nc.dram_tensor("w_fi", [K, M], sb, kind="ExternalInput")
    wco_d = nc.dram_tensor("w_co", [K, M], sb, kind="ExternalInput")
    ired_d = nc.dram_tensor("i_red", [M, H2], sb, kind="ExternalInput")
    wfc_d = nc.dram_tensor("w_fc", [K, OUT], sb, kind="ExternalInput")
    out_d = nc.dram_tensor("out", [OUT, BL], f32, kind="ExternalOutput")

    with tile.TileContext(nc) as tc:
        with (
            tc.tile_pool(name="const", bufs=1) as cpool,
            tc.tile_pool(name="state", bufs=1) as spool,
            tc.tile_pool(name="work", bufs=3) as wpool,
            tc.tile_pool(name="psum", bufs=1, space="PSUM") as pg_pool,
            tc.tile_pool(name="psum_c", bufs=1, space="PSUM") as pc_pool,
            tc.tile_pool(name="psum_fc", bufs=1, space="PSUM") as pfc_pool,
        ):
            wfi_sb = cpool.tile([K, M], sb, tag="wfi")
            wco_sb = cpool.tile([K, M], sb, tag="wco")
            ired_sb = cpool.tile([M, H2], sb, tag="ired")
            wfc_sb = cpool.tile([K, OUT], sb, tag="wfc")
            nc.sync.dma_start(wfi_sb[:], wfi_d[:])
            nc.sync.dma_start(wco_sb[:], wco_d[:])
            nc.sync.dma_start(ired_sb[:], ired_d[:])
            nc.sync.dma_start(wfc_sb[:], wfc_d[:])

            # persistent per-chain state
            # S: rows 0:50 = c, rows 64:114 = c~ scratch, pads stay zero
            S = [
                spool.tile([M, BW], sb, tag=f"S{c}", name=f"S{c}")
                for c in range(chains)
            ]
            # rhs rings: [h(0:50); x(50:55); ones(55)]
            R = [
                [
                    spool.tile([K, BW], sb, tag=f"R{c}_{i}", name=f"R{c}_{i}")
                    for i in range(NR)
                ]
                for c in range(chains)
            ]
            for c in range(chains):
                nc.vector.memset(S[c][:], 0.0)
                for i in range(NR):
                    nc.sync.dma_start(
                        R[c][i][H + IN : K, :], ones_d[:, c * BW : (c + 1) * BW]
                    )
                nc.vector.memset(R[c][0][0:H, :], 0.0)

            for t in range(T):
                for c in range(chains):
                    r = R[c][t % NR]
                    nc.sync.dma_start(
                        r[H : H + IN, :], xT_d[t][:, c * BW : (c + 1) * BW]
                    )

                    pg = pg_pool.tile(
                        [M, 2 * BW], f32, tag=f"pg{c}", name=f"pg{c}"
                    )
                    nc.tensor.matmul(
                        pg[:, 0:BW], wfi_sb[:], r[:], start=True, stop=True
                    )
                    nc.tensor.matmul(
                        pg[:, BW : 2 * BW], wco_sb[:], r[:], start=True, stop=True
                    )

                    # G: [F; I] cols 0:BW, [O; U] cols BW:2BW
                    G = wpool.tile([M, 2 * BW], sb, tag=f"G{c}", name=f"G{c}")
                    nc.scalar.activation(G[:], pg[:], AF.Sigmoid)

                    # c~ = 2U - 1 into S rows 64:114
                    nc.vector.tensor_scalar(
                        out=S[c][H2 : H2 + H, :],
                        in0=G[H2 : H2 + H, BW : 2 * BW],
                        scalar1=2.0,
                        scalar2=1.0,
                        op0=ALU.mult,
                        op1=ALU.subtract,
                    )
                    # P = [F; I] * [c; c~]   (pad rows: G*0 = 0)
                    P = wpool.tile([M, BW], sb, tag=f"P{c}", name=f"P{c}")
                    nc.vector.tensor_mul(P[:], G[:, 0:BW], S[c][:])

                    # c' = P[0:50] + P[64:114] via PE reduction
                    pc = pc_pool.tile(
                        [H2, BW], f32, tag=f"pc{c}", name=f"pc{c}"
                    )
                    nc.tensor.matmul(
                        pc[:], ired_sb[:], P[:], start=True, stop=True
                    )

                    # V = tanh(c')
                    V = wpool.tile([H, BW], sb, tag=f"V{c}", name=f"V{c}")
                    nc.scalar.activation(V[:], pc[0:H, :], AF.Tanh)

                    # h = O * V into the next rhs slot (before the c'-copy:
                    # the in-order DVE must not stall h behind it)
                    rn = R[c][(t + 1) % NR]
                    nc.vector.tensor_mul(
                        rn[0:H, :], G[0:H, BW : 2 * BW], V[:]
                    )
                    # c (sbuf) = c' — off the critical path
                    nc.vector.tensor_copy(S[c][0:H, :], pc[0:H, :])

            # final fc per chain: out = Wfc @ h + bfc (bias via ones row;
            # x rows of wfc are zero so stale x in the ring is harmless)
            for c in range(chains):
                pfc = pfc_pool.tile([OUT, BW], f32, tag="pfc", name=f"pfc{c}")
                nc.tensor.matmul(
                    pfc[:], wfc_sb[:], R[c][T % NR][:], start=True, stop=True
                )
                out_sb = wpool.tile(
                    [OUT, BW], f32, tag=f"osb{c}", name=f"osb{c}"
                )
                nc.scalar.copy(out=out_sb[:], in_=pfc[:])
                nc.sync.dma_start(out_d[:, c * BW : (c + 1) * BW], out_sb[:])

    nc.compile()
    return nc


_NC_CACHE = None


def _get_nc():
    global _NC_CACHE
    if _NC_CACHE is None:
        _NC_CACHE = _build_bass()
    return _NC_CACHE


def _in_maps(inp):
    np_dt = _np_dt()
    W_fi, W_co, Ired, W_fc = _build_weights(inp, np_dt)
    in_maps = []
    for k in range(NCORES):
        xs = inp["x"][k * BL : (k + 1) * BL]  # [BL, T, IN]
        xT = np.ascontiguousarray(np.transpose(xs, (1, 2, 0)), dtype=np_dt)
        in_maps.append(
            {
                "xT": xT,
                "ones": np.ones((1, BL), dtype=np_dt),
                "w_fi": W_fi,
                "w_co": W_co,
                "i_red": Ired,
                "w_fc": W_fc,
            }
        )
    return in_maps


def kernel(**inputs):
    from concourse.bass_utils import run_bass_kernel_spmd

    inp = {k: np.asarray(v, dtype=np.float32) for k, v in inputs.items()}
    nc = _get_nc()
    res = run_bass_kernel_spmd(nc, _in_maps(inp), list(range(NCORES)))
    outs = [res.results[k]["out"].reshape(BL, OUT) for k in range(NCORES)]
    return np.concatenate(outs, axis=0).astype(np.float32)
